# revision 7
# baseline (speedup 1.0000x reference)
"""Trainium2 Bass kernel for CARC attention processor.

Full computation:
    q/k/v = split_heads(hidden @ W{q,k,v})
    k_full = concat([k, ALPHA*K_bg], seq); v_full likewise
    scores = q @ k_full^T * scale + mask (mask zero over bg segment)
    out = softmax(scores) @ v_full  -> merge heads -> @ Wo + bo

Sharding: data-parallel over the B*H = 16 batched heads; core c owns the
adjacent head pair bh = (2c, 2c+1), both from batch b = c//4.  Projection
weight column/row slices for the pair are shipped per core; each core
returns its partial output contribution (its two heads through Wo) and the
host sums the four partials per batch and adds the bias.

Device-side layout: scores are computed transposed ([kv_chunk=128, q]) so
that the softmax denominator falls out of the PV matmul via a ones-column
appended to V (PE reduces over partitions), and probs feed the PV matmul
with no transposes anywhere.  The additive mask is applied by the vector
engine fused with the 1/sqrt(dh) scale; exp runs on the scalar engine with
the bg-segment's alpha*scale folded into the activation's scale immediate.
"""

import math

import numpy as np

import concourse.bass as bass
import concourse.tile as tile
from concourse import bacc, mybir
from concourse.bass_utils import run_bass_kernel_spmd

F32 = mybir.dt.float32

B, H, LQ, LBG, DH = 2, 8, 2048, 2048, 64
C = H * DH  # 512
ALPHA = 0.48
SCALE = 1.0 / math.sqrt(DH)
N_CORES = 8
HPC = 2  # heads per core

VE = DH + 1  # v tile width incl. ones column


def build_program(lq=LQ, lbg=LBG, c=C, nq=None):
    """Per-core program. All cores run the same NEFF on different data."""
    nq = nq or min(1024, lq)
    assert lq % 128 == 0 and lbg % 128 == 0 and c % 128 == 0 and lq % nq == 0
    n_qh = lq // nq  # q column blocks
    n_cc = c // 128  # contraction chunks for projections
    n_ts = lq // 128  # self kv / t tiles
    n_tb = lbg // 128  # bg kv tiles
    n_j = n_ts + n_tb  # kv chunks per head
    nb_mm = nq // 512 if nq >= 512 else 1  # matmuls per N block (fp32 N<=512)
    nqs = min(nq, 512)  # matmul N slice

    nc = bacc.Bacc("TRN2", target_bir_lowering=False, debug=False)

    hT = nc.dram_tensor("hT", [c, lq], F32, kind="ExternalInput")
    maskT = nc.dram_tensor("maskT", [lq, lq], F32, kind="ExternalInput")
    kbgT = nc.dram_tensor("kbgT", [HPC * DH, lbg], F32, kind="ExternalInput")
    vbg = nc.dram_tensor("vbg", [HPC, lbg, DH], F32, kind="ExternalInput")
    wq2 = nc.dram_tensor("wq2", [c, HPC * DH], F32, kind="ExternalInput")
    wk2 = nc.dram_tensor("wk2", [c, HPC * DH], F32, kind="ExternalInput")
    wv2 = nc.dram_tensor("wv2", [c, HPC * DH], F32, kind="ExternalInput")
    wo2 = nc.dram_tensor("wo2", [HPC * DH, c], F32, kind="ExternalInput")
    outp = nc.dram_tensor("outp", [lq, c], F32, kind="ExternalOutput")

    with tile.TileContext(nc) as tc:
        with tc.tile_pool(name="persist", bufs=1) as persist:
            qT = persist.tile([128, lq], F32)  # rows 0:64 head0, 64:128 head1
            kT = persist.tile([128, lq], F32)
            kbgT_sb = persist.tile([128, lbg], F32)
            vself = [
                persist.tile([128, n_ts * VE], F32, name=f"vself{h}")
                for h in range(HPC)
            ]
            vbg_sb = [
                persist.tile([128, n_tb * VE], F32, name=f"vbgsb{h}")
                for h in range(HPC)
            ]
            ctx2 = persist.tile([128, lq], F32)  # rows: [h0 d | h1 d], cols: q

            nc.sync.dma_start(out=kbgT_sb[:], in_=kbgT[:])

            # ---- Phase A: projections (qT/kT packed over heads, v natural) ----
            with (
                tc.tile_pool(name="proj_ps", bufs=2, space="PSUM") as pp,
                tc.tile_pool(name="proj_sb", bufs=1) as psb,
            ):
                wq_sb = psb.tile([128, n_cc * 128], F32)
                wk_sb = psb.tile([128, n_cc * 128], F32)
                wv_sb = psb.tile([128, n_cc * 128], F32)
                hT_sb = psb.tile([128, n_cc * lq], F32)
                for cc in range(n_cc):
                    cs = slice(cc * 128, (cc + 1) * 128)
                    nc.sync.dma_start(out=wq_sb[:, cc * 128:(cc + 1) * 128], in_=wq2[cs, :])
                    nc.sync.dma_start(out=wk_sb[:, cc * 128:(cc + 1) * 128], in_=wk2[cs, :])
                    nc.sync.dma_start(out=wv_sb[:, cc * 128:(cc + 1) * 128], in_=wv2[cs, :])
                    nc.sync.dma_start(out=hT_sb[:, cc * lq:(cc + 1) * lq], in_=hT[cs, :])

                # qT / kT: [128, lq] = (w2 chunk).T @ hT chunk, acc over chunks
                pbw = min(lq, 512)
                for w_sb, dstT in ((wq_sb, qT), (wk_sb, kT)):
                    for nb in range(lq // pbw):
                        ps = pp.tile([128, pbw], F32, tag="proj", name="ps")
                        for cc in range(n_cc):
                            nc.tensor.matmul(
                                ps[:],
                                lhsT=w_sb[:, cc * 128:(cc + 1) * 128],
                                rhs=hT_sb[:, cc * lq + nb * pbw: cc * lq + (nb + 1) * pbw],
                                start=(cc == 0),
                                stop=(cc == n_cc - 1),
                            )
                        nc.vector.tensor_copy(dstT[:, nb * pbw:(nb + 1) * pbw], ps[:])

                # v natural, both heads packed in columns; scatter into the
                # ones-column layout (whole tile pre-set to 1.0)
                for h in range(HPC):
                    nc.vector.memset(vself[h][:], 1.0)
                for tt in range(n_ts):
                    psv = pp.tile([128, HPC * DH], F32, tag="projv", name="psv")
                    for cc in range(n_cc):
                        nc.tensor.matmul(
                            psv[:],
                            lhsT=hT_sb[:, cc * lq + tt * 128: cc * lq + (tt + 1) * 128],
                            rhs=wv_sb[:, cc * 128:(cc + 1) * 128],
                            start=(cc == 0),
                            stop=(cc == n_cc - 1),
                        )
                    for h in range(HPC):
                        nc.vector.tensor_copy(
                            vself[h][:, tt * VE: tt * VE + DH],
                            psv[:, h * DH:(h + 1) * DH],
                        )

                # bg V: alpha-scaled into ones-column layout
                for h in range(HPC):
                    nc.vector.memset(vbg_sb[h][:], 1.0)
                    vtmp = psb.tile([128, n_tb * DH], F32, tag="vtmp", name="vtmp")
                    nc.sync.dma_start(
                        out=vtmp.rearrange("p (t d) -> p t d", d=DH),
                        in_=vbg[h].rearrange("(t p) d -> p t d", p=128),
                    )
                    nc.vector.tensor_scalar_mul(
                        vbg_sb[h].rearrange("p (t e) -> p t e", e=VE)[:, :, 0:DH],
                        vtmp.rearrange("p (t d) -> p t d", d=DH),
                        ALPHA,
                    )

            # ---- Phase B: attention ----
            with (
                tc.tile_pool(name="s_ps", bufs=2, space="PSUM") as sp,
                tc.tile_pool(name="c_ps", bufs=1, space="PSUM") as cp,
                tc.tile_pool(name="att_sb", bufs=3) as ab,
                tc.tile_pool(name="m_sb", bufs=3) as mb,
            ):
                ones_row = ab.tile([1, DH], F32, tag="ones", bufs=1)
                nc.vector.memset(ones_row[:], 1.0)
                for qh in range(n_qh):
                    qs = slice(qh * nq, (qh + 1) * nq)
                    Ch = [
                        cp.tile([DH + 1, nq], F32, tag=f"c{h}", name=f"ch{h}")
                        for h in range(HPC)
                    ]
                    for j in range(n_j):
                        is_self = j < n_ts
                        jj = j if is_self else j - n_ts
                        if is_self:
                            mT = mb.tile([128, nq], F32, tag="mt", name="mT")
                            nc.sync.dma_start(
                                out=mT[:], in_=maskT[jj * 128:(jj + 1) * 128, qs]
                            )
                        for h in range(HPC):
                            hs = slice(h * DH, (h + 1) * DH)
                            S = sp.tile([128, nq], F32, tag="s", name="S")
                            lT = (kT if is_self else kbgT_sb)[hs, jj * 128:(jj + 1) * 128]
                            for nb in range(nb_mm):
                                nc.tensor.matmul(
                                    S[:, nb * nqs:(nb + 1) * nqs],
                                    lhsT=lT,
                                    rhs=qT[hs, qh * nq + nb * nqs: qh * nq + (nb + 1) * nqs],
                                    start=True,
                                    stop=True,
                                )
                            P = ab.tile([128, nq], F32, tag="p", name="P")
                            if is_self:
                                M = ab.tile([128, nq], F32, tag="m", name="M")
                                nc.vector.scalar_tensor_tensor(
                                    out=M[:],
                                    in0=S[:],
                                    scalar=SCALE,
                                    in1=mT[:],
                                    op0=mybir.AluOpType.mult,
                                    op1=mybir.AluOpType.add,
                                )
                                nc.scalar.activation(
                                    P[:], M[:], mybir.ActivationFunctionType.Exp
                                )
                            else:
                                nc.scalar.activation(
                                    P[:], S[:], mybir.ActivationFunctionType.Exp,
                                    scale=ALPHA * SCALE,
                                )
                            vext = (vself if is_self else vbg_sb)[h][
                                :, jj * VE:(jj + 1) * VE
                            ]
                            for nb in range(nb_mm):
                                nc.tensor.matmul(
                                    Ch[h][:, nb * nqs:(nb + 1) * nqs],
                                    lhsT=vext,
                                    rhs=P[:, nb * nqs:(nb + 1) * nqs],
                                    start=(j == 0),
                                    stop=(j == n_j - 1),
                                )
                    # normalize: ctx2[h] = Ch[h][0:DH] * (1/den) with den = row DH
                    for h in range(HPC):
                        recip = ab.tile([1, nq], F32, tag="recip", name="recip")
                        nc.vector.reciprocal(recip[:], Ch[h][DH:DH + 1, :])
                        rbp = sp.tile([DH, nq], F32, tag="s", name="rbp")
                        for nb in range(nb_mm):
                            nc.tensor.matmul(
                                rbp[:, nb * nqs:(nb + 1) * nqs],
                                lhsT=ones_row[:],
                                rhs=recip[:, nb * nqs:(nb + 1) * nqs],
                                start=True,
                                stop=True,
                            )
                        rb = ab.tile([DH, nq], F32, tag="rb", name="rb")
                        nc.vector.tensor_copy(rb[:], rbp[:])
                        nc.vector.tensor_tensor(
                            out=ctx2[h * DH:(h + 1) * DH, qs],
                            in0=Ch[h][0:DH, :],
                            in1=rb[:],
                            op=mybir.AluOpType.mult,
                        )

            # ---- Phase C: output projection (partial; host sums heads) ----
            with (
                tc.tile_pool(name="wo_ps", bufs=2, space="PSUM") as wp,
                tc.tile_pool(name="wo_sb", bufs=2) as ob,
                tc.tile_pool(name="wo_w", bufs=1) as wwp,
            ):
                wo_sb = wwp.tile([HPC * DH, c], F32)
                nc.sync.dma_start(out=wo_sb[:], in_=wo2[:])
                for tt in range(n_ts):
                    for nb in range(c // 512 if c >= 512 else 1):
                        ncol = min(c, 512)
                        po = wp.tile([128, ncol], F32, tag="wo", name="po")
                        nc.tensor.matmul(
                            po[:],
                            lhsT=ctx2[:, tt * 128:(tt + 1) * 128],
                            rhs=wo_sb[:, nb * ncol:(nb + 1) * ncol],
                            start=True,
                            stop=True,
                        )
                        osb = ob.tile([128, ncol], F32, tag="ob", name="osb")
                        nc.vector.tensor_copy(osb[:], po[:])
                        nc.sync.dma_start(
                            out=outp[tt * 128:(tt + 1) * 128, nb * ncol:(nb + 1) * ncol],
                            in_=osb[:],
                        )

    nc.compile()
    return nc


_NC_CACHE = {}


def _get_nc(key=(LQ, LBG, C)):
    if key not in _NC_CACHE:
        _NC_CACHE[key] = build_program(*key)
    return _NC_CACHE[key]


def make_in_maps(hidden_states, attention_mask, K_bg, V_bg, Wq, Wk, Wv, Wo):
    f = lambda a: np.ascontiguousarray(np.asarray(a, dtype=np.float32))
    hiddenT = [f(np.asarray(hidden_states)[b].T) for b in range(B)]
    maskT = [f(np.asarray(attention_mask)[b].T) for b in range(B)]
    K_bg, V_bg = np.asarray(K_bg), np.asarray(V_bg)
    Wq, Wk, Wv, Wo = map(np.asarray, (Wq, Wk, Wv, Wo))
    in_maps = []
    for core in range(N_CORES):
        bh0 = HPC * core
        b = bh0 // H
        h0 = bh0 % H
        cs = slice(h0 * DH, (h0 + HPC) * DH)
        in_maps.append({
            "hT": hiddenT[b],
            "maskT": maskT[b],
            "kbgT": f(K_bg[bh0:bh0 + HPC].transpose(0, 2, 1).reshape(HPC * DH, LBG)),
            "vbg": f(V_bg[bh0:bh0 + HPC]),
            "wq2": f(Wq[:, cs]),
            "wk2": f(Wk[:, cs]),
            "wv2": f(Wv[:, cs]),
            "wo2": f(Wo[cs, :]),
        })
    return in_maps


def _run(in_maps, trace=False, **kw):
    nc = _get_nc()
    return run_bass_kernel_spmd(nc, in_maps, list(range(N_CORES)), trace=trace, **kw)


def kernel(hidden_states, attention_mask, K_bg, V_bg, Wq, Wk, Wv, Wo, bo):
    in_maps = make_in_maps(
        hidden_states, attention_mask, K_bg, V_bg, Wq, Wk, Wv, Wo
    )
    res = _run(in_maps)
    out = np.zeros((B, LQ, C), np.float32)
    for core in range(N_CORES):
        out[core // (N_CORES // B)] += res.results[core]["outp"]
    out += np.asarray(bo, dtype=np.float32)
    return out


# revision 14
# speedup vs baseline: 1.7980x; 1.7980x over previous
"""Trainium2 Bass kernel for CARC attention processor.

Full computation:
    q/k/v = split_heads(hidden @ W{q,k,v})
    k_full = concat([k, ALPHA*K_bg], seq); v_full likewise
    scores = q @ k_full^T * scale + mask (mask zero over bg segment)
    out = softmax(scores) @ v_full  -> merge heads -> @ Wo + bo

Sharding: data-parallel over the B*H = 16 batched heads; core c owns the
adjacent head pair bh = (2c, 2c+1), both from batch b = c//4.  Projection
weight column/row slices for the pair are shipped per core; each core
returns its partial output contribution (its two heads through Wo) and the
host sums the four partials per batch and adds the bias.

Device-side layout: scores are computed transposed ([kv_chunk=128, q]) so
that the softmax denominator falls out of the PV matmul via a ones-column
appended to V (PE reduces over partitions), and probs feed the PV matmul
with no transposes anywhere.  The additive mask is applied by the vector
engine fused with the 1/sqrt(dh) scale; exp runs on the scalar engine with
the bg-segment's alpha*scale folded into the activation's scale immediate.
Matmul operands are bf16 (fp32 is double-pass half-rate on trn2 PE); the
masked scores stay fp32 into exp, accumulation is fp32 in PSUM, and the
softmax normalization (reciprocal + broadcast) is exact fp32.
"""

import math

import numpy as np

import concourse.bass as bass  # noqa: F401
import concourse.tile as tile
from concourse import bacc, mybir
from concourse.bass_utils import run_bass_kernel_spmd

F32 = mybir.dt.float32
BF16 = mybir.dt.bfloat16

B, H, LQ, LBG, DH = 2, 8, 2048, 2048, 64
C = H * DH  # 512
ALPHA = 0.48
SCALE = 1.0 / math.sqrt(DH)
N_CORES = 8
HPC = 2  # heads per core

VE = DH + 1  # v tile width incl. ones column


def build_program(lq=LQ, lbg=LBG, c=C, nq=None):
    """Per-core program. All cores run the same NEFF on different data."""
    nq = nq or min(1024, lq)
    assert lq % 128 == 0 and lbg % 128 == 0 and c % 128 == 0 and lq % nq == 0
    n_qh = lq // nq  # q column blocks
    n_cc = c // 128  # contraction chunks for projections
    n_ts = lq // 128  # self kv / t tiles
    n_tb = lbg // 128  # bg kv tiles
    n_j = n_ts + n_tb  # kv chunks per head
    assert nq <= 1024  # bf16 moving-operand limit

    nc = bacc.Bacc("TRN2", target_bir_lowering=False, debug=False)

    hT = nc.dram_tensor("hT", [c, lq], F32, kind="ExternalInput")
    maskT = nc.dram_tensor("maskT", [lq, lq], F32, kind="ExternalInput")
    kbgT = nc.dram_tensor("kbgT", [HPC * DH, lbg], F32, kind="ExternalInput")
    vbg = nc.dram_tensor("vbg", [HPC, lbg, DH], F32, kind="ExternalInput")
    wq2 = nc.dram_tensor("wq2", [c, HPC * DH], F32, kind="ExternalInput")
    wk2 = nc.dram_tensor("wk2", [c, HPC * DH], F32, kind="ExternalInput")
    wv2 = nc.dram_tensor("wv2", [c, HPC * DH], F32, kind="ExternalInput")
    wo2 = nc.dram_tensor("wo2", [HPC * DH, c], F32, kind="ExternalInput")
    outp = nc.dram_tensor("outp", [lq, c], F32, kind="ExternalOutput")

    with tile.TileContext(nc) as tc:
        with tc.tile_pool(name="persist", bufs=1) as persist:
            qT = persist.tile([128, lq], BF16)  # rows 0:64 head0, 64:128 head1
            kT = persist.tile([128, lq], BF16)
            kbgT_sb = persist.tile([128, lbg], BF16)
            vself = [
                persist.tile([128, n_ts * VE], BF16, name=f"vself{h}")
                for h in range(HPC)
            ]
            vbg_sb = [
                persist.tile([128, n_tb * VE], BF16, name=f"vbgsb{h}")
                for h in range(HPC)
            ]
            ctx2 = persist.tile([128, lq], BF16)  # rows: [h0 d | h1 d], cols: q

            # ---- Phase A: projections (qT/kT packed over heads, v natural) ----
            with (
                tc.tile_pool(name="proj_ps", bufs=2, space="PSUM") as pp,
                tc.tile_pool(name="proj_sb", bufs=1) as psb,
            ):
                ktmp = psb.tile([128, lbg], F32)
                nc.sync.dma_start(out=ktmp[:], in_=kbgT[:])
                nc.vector.tensor_copy(kbgT_sb[:], ktmp[:])

                wq_sb = psb.tile([128, n_cc * 128], BF16)
                wk_sb = psb.tile([128, n_cc * 128], BF16)
                wv_sb = psb.tile([128, n_cc * 128], BF16)
                hT_sb = psb.tile([128, n_cc * lq], BF16)
                wtmp = psb.tile([128, n_cc * 128], F32)
                htmp = psb.tile([128, n_cc * lq], F32)
                for w_dram, w_bf in ((wq2, wq_sb), (wk2, wk_sb), (wv2, wv_sb)):
                    nc.sync.dma_start(
                        out=wtmp.rearrange("p (cc x) -> p cc x", x=128),
                        in_=w_dram.rearrange("(cc p) x -> p cc x", p=128),
                    )
                    nc.vector.tensor_copy(w_bf[:], wtmp[:])
                for cc in range(n_cc):
                    nc.sync.dma_start(
                        out=htmp[:, cc * lq:(cc + 1) * lq],
                        in_=hT[cc * 128:(cc + 1) * 128, :],
                    )
                nc.vector.tensor_copy(hT_sb[:], htmp[:])

                # qT / kT: [128, lq] = (w2 chunk).T @ hT chunk, acc over chunks
                pbw = min(lq, 512)
                for w_sb, dstT in ((wq_sb, qT), (wk_sb, kT)):
                    for nb in range(lq // pbw):
                        ps = pp.tile([128, pbw], F32, tag="proj", name="ps")
                        for cc in range(n_cc):
                            nc.tensor.matmul(
                                ps[:],
                                lhsT=w_sb[:, cc * 128:(cc + 1) * 128],
                                rhs=hT_sb[:, cc * lq + nb * pbw: cc * lq + (nb + 1) * pbw],
                                start=(cc == 0),
                                stop=(cc == n_cc - 1),
                            )
                        nc.vector.tensor_copy(dstT[:, nb * pbw:(nb + 1) * pbw], ps[:])

                # v natural, both heads packed in columns; scatter into the
                # ones-column layout (whole tile pre-set to 1.0)
                for h in range(HPC):
                    nc.vector.memset(vself[h][:], 1.0)
                for tt in range(n_ts):
                    psv = pp.tile([128, HPC * DH], F32, tag="projv", name="psv")
                    for cc in range(n_cc):
                        nc.tensor.matmul(
                            psv[:],
                            lhsT=hT_sb[:, cc * lq + tt * 128: cc * lq + (tt + 1) * 128],
                            rhs=wv_sb[:, cc * 128:(cc + 1) * 128],
                            start=(cc == 0),
                            stop=(cc == n_cc - 1),
                        )
                    for h in range(HPC):
                        nc.vector.tensor_copy(
                            vself[h][:, tt * VE: tt * VE + DH],
                            psv[:, h * DH:(h + 1) * DH],
                        )

                # bg V: alpha-scaled into ones-column layout
                for h in range(HPC):
                    nc.vector.memset(vbg_sb[h][:], 1.0)
                    vtmp = psb.tile([128, n_tb * DH], F32, tag="vtmp", name="vtmp")
                    nc.sync.dma_start(
                        out=vtmp.rearrange("p (t d) -> p t d", d=DH),
                        in_=vbg[h].rearrange("(t p) d -> p t d", p=128),
                    )
                    nc.vector.tensor_scalar_mul(
                        vbg_sb[h].rearrange("p (t e) -> p t e", e=VE)[:, :, 0:DH],
                        vtmp.rearrange("p (t d) -> p t d", d=DH),
                        ALPHA,
                    )

            # ---- Phase B: attention ----
            with (
                tc.tile_pool(name="s_ps", bufs=2, space="PSUM") as sp,
                tc.tile_pool(name="c_ps", bufs=1, space="PSUM") as cp,
                tc.tile_pool(name="att_sb", bufs=3) as ab,
                tc.tile_pool(name="m_sb", bufs=3) as mb,
            ):
                ones_row = ab.tile([1, DH], F32, tag="ones", bufs=1)
                nc.vector.memset(ones_row[:], 1.0)
                for qh in range(n_qh):
                    qs = slice(qh * nq, (qh + 1) * nq)
                    Ch = [
                        cp.tile([DH + 1, nq], F32, tag=f"c{h}", name=f"ch{h}")
                        for h in range(HPC)
                    ]
                    for j in range(n_j):
                        is_self = j < n_ts
                        jj = j if is_self else j - n_ts
                        if is_self:
                            mT = mb.tile([128, nq], F32, tag="mt", name="mT")
                            nc.sync.dma_start(
                                out=mT[:], in_=maskT[jj * 128:(jj + 1) * 128, qs]
                            )
                        for h in range(HPC):
                            hs = slice(h * DH, (h + 1) * DH)
                            S = sp.tile([128, nq], F32, tag="s", name="S")
                            lT = (kT if is_self else kbgT_sb)[hs, jj * 128:(jj + 1) * 128]
                            nw = min(nq, 512)
                            for nb in range(nq // nw):
                                ns = slice(nb * nw, (nb + 1) * nw)
                                nc.tensor.matmul(
                                    S[:, ns], lhsT=lT,
                                    rhs=qT[hs, qh * nq + nb * nw: qh * nq + (nb + 1) * nw],
                                    start=True, stop=True,
                                )
                            P = ab.tile([128, nq], BF16, tag="p", name="P")
                            if is_self:
                                M = ab.tile([128, nq], F32, tag="m", name="M")
                                nc.vector.scalar_tensor_tensor(
                                    out=M[:],
                                    in0=S[:],
                                    scalar=SCALE,
                                    in1=mT[:],
                                    op0=mybir.AluOpType.mult,
                                    op1=mybir.AluOpType.add,
                                )
                                nc.scalar.activation(
                                    P[:], M[:], mybir.ActivationFunctionType.Exp
                                )
                            else:
                                nc.scalar.activation(
                                    P[:], S[:], mybir.ActivationFunctionType.Exp,
                                    scale=ALPHA * SCALE,
                                )
                            vext = (vself if is_self else vbg_sb)[h][
                                :, jj * VE:(jj + 1) * VE
                            ]
                            for nb in range(nq // nw):
                                ns = slice(nb * nw, (nb + 1) * nw)
                                nc.tensor.matmul(
                                    Ch[h][:, ns], lhsT=vext, rhs=P[:, ns],
                                    start=(j == 0), stop=(j == n_j - 1),
                                )
                    # normalize: ctx2[h] = Ch[h][0:DH] * (1/den) with den = row DH
                    for h in range(HPC):
                        recip = ab.tile([1, nq], F32, tag="recip", name="recip")
                        nc.vector.reciprocal(recip[:], Ch[h][DH:DH + 1, :])
                        rbp = sp.tile([DH, nq], F32, tag="s", name="rbp")
                        bw = min(nq, 512)
                        for nb in range(nq // bw):
                            nc.tensor.matmul(
                                rbp[:, nb * bw:(nb + 1) * bw],
                                lhsT=ones_row[:],
                                rhs=recip[:, nb * bw:(nb + 1) * bw],
                                start=True,
                                stop=True,
                            )
                        rb = ab.tile([DH, nq], F32, tag="rb", name="rb")
                        nc.vector.tensor_copy(rb[:], rbp[:])
                        nc.vector.tensor_tensor(
                            out=ctx2[h * DH:(h + 1) * DH, qs],
                            in0=Ch[h][0:DH, :],
                            in1=rb[:],
                            op=mybir.AluOpType.mult,
                        )

            # ---- Phase C: output projection (partial; host sums heads) ----
            with (
                tc.tile_pool(name="wo_ps", bufs=2, space="PSUM") as wp,
                tc.tile_pool(name="wo_sb", bufs=2) as ob,
                tc.tile_pool(name="wo_w", bufs=1) as wwp,
            ):
                wotmp = wwp.tile([HPC * DH, c], F32)
                nc.sync.dma_start(out=wotmp[:], in_=wo2[:])
                wo_sb = wwp.tile([HPC * DH, c], BF16)
                nc.vector.tensor_copy(wo_sb[:], wotmp[:])
                ncol = min(c, 512)
                for tt in range(n_ts):
                    for nb in range(c // ncol):
                        po = wp.tile([128, ncol], F32, tag="wo", name="po")
                        nc.tensor.matmul(
                            po[:],
                            lhsT=ctx2[:, tt * 128:(tt + 1) * 128],
                            rhs=wo_sb[:, nb * ncol:(nb + 1) * ncol],
                            start=True,
                            stop=True,
                        )
                        osb = ob.tile([128, ncol], F32, tag="ob", name="osb")
                        nc.vector.tensor_copy(osb[:], po[:])
                        nc.sync.dma_start(
                            out=outp[tt * 128:(tt + 1) * 128, nb * ncol:(nb + 1) * ncol],
                            in_=osb[:],
                        )

    nc.compile()
    return nc


_NC_CACHE = {}


def _get_nc(key=(LQ, LBG, C)):
    if key not in _NC_CACHE:
        _NC_CACHE[key] = build_program(*key)
    return _NC_CACHE[key]


def make_in_maps(hidden_states, attention_mask, K_bg, V_bg, Wq, Wk, Wv, Wo):
    f = lambda a: np.ascontiguousarray(np.asarray(a, dtype=np.float32))
    hiddenT = [f(np.asarray(hidden_states)[b].T) for b in range(B)]
    maskT = [f(np.asarray(attention_mask)[b].T) for b in range(B)]
    K_bg, V_bg = np.asarray(K_bg), np.asarray(V_bg)
    Wq, Wk, Wv, Wo = map(np.asarray, (Wq, Wk, Wv, Wo))
    in_maps = []
    for core in range(N_CORES):
        bh0 = HPC * core
        b = bh0 // H
        h0 = bh0 % H
        cs = slice(h0 * DH, (h0 + HPC) * DH)
        in_maps.append({
            "hT": hiddenT[b],
            "maskT": maskT[b],
            "kbgT": f(K_bg[bh0:bh0 + HPC].transpose(0, 2, 1).reshape(HPC * DH, LBG)),
            "vbg": f(V_bg[bh0:bh0 + HPC]),
            "wq2": f(Wq[:, cs]),
            "wk2": f(Wk[:, cs]),
            "wv2": f(Wv[:, cs]),
            "wo2": f(Wo[cs, :]),
        })
    return in_maps


def _run(in_maps, trace=False, **kw):
    nc = _get_nc()
    return run_bass_kernel_spmd(nc, in_maps, list(range(N_CORES)), trace=trace, **kw)


def kernel(hidden_states, attention_mask, K_bg, V_bg, Wq, Wk, Wv, Wo, bo):
    in_maps = make_in_maps(
        hidden_states, attention_mask, K_bg, V_bg, Wq, Wk, Wv, Wo
    )
    res = _run(in_maps)
    out = np.zeros((B, LQ, C), np.float32)
    for core in range(N_CORES):
        out[core // (N_CORES // B)] += res.results[core]["outp"]
    out += np.asarray(bo, dtype=np.float32)
    return out


# revision 17
# speedup vs baseline: 1.9642x; 1.0924x over previous
"""Trainium2 Bass kernel for CARC attention processor.

Full computation:
    q/k/v = split_heads(hidden @ W{q,k,v})
    k_full = concat([k, ALPHA*K_bg], seq); v_full likewise
    scores = q @ k_full^T * scale + mask (mask zero over bg segment)
    out = softmax(scores) @ v_full  -> merge heads -> @ Wo + bo

Sharding: data-parallel over the B*H = 16 batched heads; core c owns the
adjacent head pair bh = (2c, 2c+1), both from batch b = c//4.  Projection
weight column/row slices for the pair are shipped per core; each core
returns its partial output contribution (its two heads through Wo) and the
host sums the four partials per batch and adds the bias.

Device-side layout: scores are computed transposed ([kv_chunk=128, q]) so
that the softmax denominator falls out of the PV matmul via a ones-column
appended to V (PE reduces over partitions), and probs feed the PV matmul
with no transposes anywhere.  The additive mask is applied by the vector
engine fused with the 1/sqrt(dh) scale; exp runs on the scalar engine with
the bg-segment's alpha*scale folded into the activation's scale immediate.
Matmul operands are bf16 (fp32 is double-pass half-rate on trn2 PE); the
masked scores stay fp32 into exp, accumulation is fp32 in PSUM, and the
softmax normalization (reciprocal + broadcast) is exact fp32.
"""

import math

import numpy as np

import concourse.bass as bass  # noqa: F401
import concourse.tile as tile
from concourse import bacc, mybir
from concourse.bass_utils import run_bass_kernel_spmd

F32 = mybir.dt.float32
BF16 = mybir.dt.bfloat16

B, H, LQ, LBG, DH = 2, 8, 2048, 2048, 64
C = H * DH  # 512
ALPHA = 0.48
SCALE = 1.0 / math.sqrt(DH)
N_CORES = 8
HPC = 2  # heads per core

VE = DH + 1  # v tile width incl. ones column


def build_program(lq=LQ, lbg=LBG, c=C, nq=None):
    """Per-core program. All cores run the same NEFF on different data."""
    nq = nq or min(1024, lq)
    assert lq % 128 == 0 and lbg % 128 == 0 and c % 128 == 0 and lq % nq == 0
    n_qh = lq // nq  # q column blocks
    n_cc = c // 128  # contraction chunks for projections
    n_ts = lq // 128  # self kv / t tiles
    n_tb = lbg // 128  # bg kv tiles
    n_j = n_ts + n_tb  # kv chunks per head
    assert nq <= 1024  # bf16 moving-operand limit

    nc = bacc.Bacc("TRN2", target_bir_lowering=False, debug=False)

    hT = nc.dram_tensor("hT", [c, lq], F32, kind="ExternalInput")
    maskT = nc.dram_tensor("maskT", [lq, lq], F32, kind="ExternalInput")
    kbgT = nc.dram_tensor("kbgT", [HPC * DH, lbg], F32, kind="ExternalInput")
    vbg = nc.dram_tensor("vbg", [HPC, lbg, DH], F32, kind="ExternalInput")
    wq2 = nc.dram_tensor("wq2", [c, HPC * DH], F32, kind="ExternalInput")
    wk2 = nc.dram_tensor("wk2", [c, HPC * DH], F32, kind="ExternalInput")
    wv2 = nc.dram_tensor("wv2", [c, HPC * DH], F32, kind="ExternalInput")
    wo2 = nc.dram_tensor("wo2", [HPC * DH, c], F32, kind="ExternalInput")
    outp = nc.dram_tensor("outp", [lq, c], F32, kind="ExternalOutput")

    with tile.TileContext(nc) as tc:
        with tc.tile_pool(name="persist", bufs=1) as persist:
            qT = persist.tile([128, lq], BF16)  # rows 0:64 head0, 64:128 head1
            kT = persist.tile([128, lq], BF16)
            kbgT_sb = persist.tile([128, lbg], BF16)
            vself = [
                persist.tile([128, n_ts * VE], BF16, name=f"vself{h}")
                for h in range(HPC)
            ]
            vbg_sb = [
                persist.tile([128, n_tb * VE], BF16, name=f"vbgsb{h}")
                for h in range(HPC)
            ]
            ctx2 = persist.tile([128, lq], BF16)  # rows: [h0 d | h1 d], cols: q

            # ---- Phase A: projections (qT/kT packed over heads, v natural) ----
            with (
                tc.tile_pool(name="proj_ps", bufs=2, space="PSUM") as pp,
                tc.tile_pool(name="proj_sb", bufs=1) as psb,
            ):
                ktmp = psb.tile([128, lbg], F32)
                nc.sync.dma_start(out=ktmp[:], in_=kbgT[:])
                nc.vector.tensor_copy(kbgT_sb[:], ktmp[:])

                wq_sb = psb.tile([128, n_cc * 128], BF16)
                wk_sb = psb.tile([128, n_cc * 128], BF16)
                wv_sb = psb.tile([128, n_cc * 128], BF16)
                hT_sb = psb.tile([128, n_cc * lq], BF16)
                wtmp = psb.tile([128, n_cc * 128], F32)
                htmp = psb.tile([128, n_cc * lq], F32)
                for w_dram, w_bf in ((wq2, wq_sb), (wk2, wk_sb), (wv2, wv_sb)):
                    nc.sync.dma_start(
                        out=wtmp.rearrange("p (cc x) -> p cc x", x=128),
                        in_=w_dram.rearrange("(cc p) x -> p cc x", p=128),
                    )
                    nc.vector.tensor_copy(w_bf[:], wtmp[:])
                for cc in range(n_cc):
                    nc.sync.dma_start(
                        out=htmp[:, cc * lq:(cc + 1) * lq],
                        in_=hT[cc * 128:(cc + 1) * 128, :],
                    )
                nc.vector.tensor_copy(hT_sb[:], htmp[:])

                # qT / kT: [128, lq] = (w2 chunk).T @ hT chunk, acc over chunks
                pbw = min(lq, 512)
                for w_sb, dstT in ((wq_sb, qT), (wk_sb, kT)):
                    for nb in range(lq // pbw):
                        ps = pp.tile([128, pbw], F32, tag="proj", name="ps")
                        for cc in range(n_cc):
                            nc.tensor.matmul(
                                ps[:],
                                lhsT=w_sb[:, cc * 128:(cc + 1) * 128],
                                rhs=hT_sb[:, cc * lq + nb * pbw: cc * lq + (nb + 1) * pbw],
                                start=(cc == 0),
                                stop=(cc == n_cc - 1),
                            )
                        nc.vector.tensor_copy(dstT[:, nb * pbw:(nb + 1) * pbw], ps[:])

                # v natural, both heads packed in columns; scatter into the
                # ones-column layout (whole tile pre-set to 1.0)
                for h in range(HPC):
                    nc.vector.memset(vself[h][:], 1.0)
                for tt in range(n_ts):
                    psv = pp.tile([128, HPC * DH], F32, tag="projv", name="psv")
                    for cc in range(n_cc):
                        nc.tensor.matmul(
                            psv[:],
                            lhsT=hT_sb[:, cc * lq + tt * 128: cc * lq + (tt + 1) * 128],
                            rhs=wv_sb[:, cc * 128:(cc + 1) * 128],
                            start=(cc == 0),
                            stop=(cc == n_cc - 1),
                        )
                    for h in range(HPC):
                        nc.vector.tensor_copy(
                            vself[h][:, tt * VE: tt * VE + DH],
                            psv[:, h * DH:(h + 1) * DH],
                        )

                # bg V: alpha-scaled into ones-column layout
                for h in range(HPC):
                    nc.vector.memset(vbg_sb[h][:], 1.0)
                    vtmp = psb.tile([128, n_tb * DH], F32, tag="vtmp", name="vtmp")
                    nc.sync.dma_start(
                        out=vtmp.rearrange("p (t d) -> p t d", d=DH),
                        in_=vbg[h].rearrange("(t p) d -> p t d", p=128),
                    )
                    nc.vector.tensor_scalar_mul(
                        vbg_sb[h].rearrange("p (t e) -> p t e", e=VE)[:, :, 0:DH],
                        vtmp.rearrange("p (t d) -> p t d", d=DH),
                        ALPHA,
                    )

            # ---- Phase B: attention ----
            with (
                tc.tile_pool(name="s_ps", bufs=2, space="PSUM") as sp,
                tc.tile_pool(name="c_ps", bufs=1, space="PSUM") as cp,
                tc.tile_pool(name="att_sb", bufs=3) as ab,
                tc.tile_pool(name="m_sb", bufs=4) as mb,
                tc.tile_pool(name="dram_p", bufs=2, space="DRAM") as dp,
            ):
                for qh in range(n_qh):
                    qs = slice(qh * nq, (qh + 1) * nq)
                    Ch = [
                        cp.tile([DH + 1, nq], F32, tag=f"c{h}", name=f"ch{h}")
                        for h in range(HPC)
                    ]
                    for j in range(n_j):
                        is_self = j < n_ts
                        jj = j if is_self else j - n_ts
                        if is_self:
                            mT = mb.tile([128, nq], F32, tag="mt", name="mT")
                            nc.sync.dma_start(
                                out=mT[:], in_=maskT[jj * 128:(jj + 1) * 128, qs]
                            )
                        for h in range(HPC):
                            hs = slice(h * DH, (h + 1) * DH)
                            S = sp.tile([128, nq], F32, tag="s", name="S")
                            lT = (kT if is_self else kbgT_sb)[hs, jj * 128:(jj + 1) * 128]
                            nw = min(nq, 512)
                            for nb in range(nq // nw):
                                ns = slice(nb * nw, (nb + 1) * nw)
                                nc.tensor.matmul(
                                    S[:, ns], lhsT=lT,
                                    rhs=qT[hs, qh * nq + nb * nw: qh * nq + (nb + 1) * nw],
                                    start=True, stop=True,
                                )
                            P = ab.tile([128, nq], BF16, tag="p", name="P", bufs=8)
                            if is_self:
                                M = ab.tile([128, nq], F32, tag="m", name="M", bufs=4)
                                nc.vector.scalar_tensor_tensor(
                                    out=M[:],
                                    in0=S[:],
                                    scalar=SCALE,
                                    in1=mT[:],
                                    op0=mybir.AluOpType.mult,
                                    op1=mybir.AluOpType.add,
                                )
                                nc.scalar.activation(
                                    P[:], M[:], mybir.ActivationFunctionType.Exp
                                )
                            else:
                                nc.scalar.activation(
                                    P[:], S[:], mybir.ActivationFunctionType.Exp,
                                    scale=ALPHA * SCALE,
                                )
                            vext = (vself if is_self else vbg_sb)[h][
                                :, jj * VE:(jj + 1) * VE
                            ]
                            for nb in range(nq // nw):
                                ns = slice(nb * nw, (nb + 1) * nw)
                                nc.tensor.matmul(
                                    Ch[h][:, ns], lhsT=vext, rhs=P[:, ns],
                                    start=(j == 0), stop=(j == n_j - 1),
                                )
                    # normalize: ctx2[h] = Ch[h][0:DH] * (1/den) with den = row DH
                    # (recip broadcast along partitions via a DRAM round-trip;
                    # DMA reads from DRAM may broadcast, SBUF reads may not)
                    for h in range(HPC):
                        recip = ab.tile([1, nq], F32, tag="recip", name="recip")
                        nc.vector.reciprocal(recip[:], Ch[h][DH:DH + 1, :])
                        rdram = dp.tile([1, nq], F32, tag="rd", name="rdram")
                        nc.sync.dma_start(out=rdram[:], in_=recip[:])
                        rb = ab.tile([DH, nq], F32, tag="rb", name="rb")
                        nc.sync.dma_start(
                            out=rb[:], in_=rdram.to_broadcast((DH, nq))
                        )
                        nc.vector.tensor_tensor(
                            out=ctx2[h * DH:(h + 1) * DH, qs],
                            in0=Ch[h][0:DH, :],
                            in1=rb[:],
                            op=mybir.AluOpType.mult,
                        )

            # ---- Phase C: output projection (partial; host sums heads) ----
            with (
                tc.tile_pool(name="wo_ps", bufs=2, space="PSUM") as wp,
                tc.tile_pool(name="wo_sb", bufs=2) as ob,
                tc.tile_pool(name="wo_w", bufs=1) as wwp,
            ):
                wotmp = wwp.tile([HPC * DH, c], F32)
                nc.sync.dma_start(out=wotmp[:], in_=wo2[:])
                wo_sb = wwp.tile([HPC * DH, c], BF16)
                nc.vector.tensor_copy(wo_sb[:], wotmp[:])
                ncol = min(c, 512)
                for tt in range(n_ts):
                    for nb in range(c // ncol):
                        po = wp.tile([128, ncol], F32, tag="wo", name="po")
                        nc.tensor.matmul(
                            po[:],
                            lhsT=ctx2[:, tt * 128:(tt + 1) * 128],
                            rhs=wo_sb[:, nb * ncol:(nb + 1) * ncol],
                            start=True,
                            stop=True,
                        )
                        osb = ob.tile([128, ncol], F32, tag="ob", name="osb")
                        nc.vector.tensor_copy(osb[:], po[:])
                        nc.sync.dma_start(
                            out=outp[tt * 128:(tt + 1) * 128, nb * ncol:(nb + 1) * ncol],
                            in_=osb[:],
                        )

    nc.compile()
    return nc


_NC_CACHE = {}


def _get_nc(key=(LQ, LBG, C)):
    if key not in _NC_CACHE:
        _NC_CACHE[key] = build_program(*key)
    return _NC_CACHE[key]


def make_in_maps(hidden_states, attention_mask, K_bg, V_bg, Wq, Wk, Wv, Wo):
    f = lambda a: np.ascontiguousarray(np.asarray(a, dtype=np.float32))
    hiddenT = [f(np.asarray(hidden_states)[b].T) for b in range(B)]
    maskT = [f(np.asarray(attention_mask)[b].T) for b in range(B)]
    K_bg, V_bg = np.asarray(K_bg), np.asarray(V_bg)
    Wq, Wk, Wv, Wo = map(np.asarray, (Wq, Wk, Wv, Wo))
    in_maps = []
    for core in range(N_CORES):
        bh0 = HPC * core
        b = bh0 // H
        h0 = bh0 % H
        cs = slice(h0 * DH, (h0 + HPC) * DH)
        in_maps.append({
            "hT": hiddenT[b],
            "maskT": maskT[b],
            "kbgT": f(K_bg[bh0:bh0 + HPC].transpose(0, 2, 1).reshape(HPC * DH, LBG)),
            "vbg": f(V_bg[bh0:bh0 + HPC]),
            "wq2": f(Wq[:, cs]),
            "wk2": f(Wk[:, cs]),
            "wv2": f(Wv[:, cs]),
            "wo2": f(Wo[cs, :]),
        })
    return in_maps


def _run(in_maps, trace=False, **kw):
    nc = _get_nc()
    return run_bass_kernel_spmd(nc, in_maps, list(range(N_CORES)), trace=trace, **kw)


def kernel(hidden_states, attention_mask, K_bg, V_bg, Wq, Wk, Wv, Wo, bo):
    in_maps = make_in_maps(
        hidden_states, attention_mask, K_bg, V_bg, Wq, Wk, Wv, Wo
    )
    res = _run(in_maps)
    out = np.zeros((B, LQ, C), np.float32)
    for core in range(N_CORES):
        out[core // (N_CORES // B)] += res.results[core]["outp"]
    out += np.asarray(bo, dtype=np.float32)
    return out
